# revision 45
# baseline (speedup 1.0000x reference)
"""Trainium2 Bass kernel for nn_MultiHeadAttention_79723182949055.

Math (per reference):
    r1 = einsum('bmp,kpd->bmkd', y, Lam_x)          # key proj
    s  = einsum('bnq,kqd->bnkd', y, Lam_y)          # query proj
    S  = einsum('bmkd,bnkd->kbmn', r1, s) + mask    # scores
    A  = softmax(S / sqrt(D), axis=m)
    w  = einsum('bmp,kpd->bmkd', y, Th_x)           # value proj
    U  = einsum('kbmn,bmkd->bnkd', A, w)            # aggregation
    out= einsum('bnkd,kqd->bnq', U, Th_y)           # out proj

Sharding: 8 cores; core c handles batch b = c//4 and heads 4*(c%4) .. +4.
Each core computes a partial out[b] (sum over its 4 heads); host sums the
4 partials per batch.

Device algorithm per core (fp16 operands, fp32 PSUM accumulation):
  - yT [P=1024, M=2048] host-pretransposed fp16 input.
  - r1z[j] [128, M]: rows po:po+64 = r1T of head j, other rows zero, so a
    full 128-partition score contraction picks out head j only.
    sT[hp] [128, N] holds both heads of pair hp.
  - w4[m] [128, 4*128] fp16: head j cols j*128+0:64 = w, col 64 = ones
    (colsum for free), rest zero -- 128-col stationary keeps fast
    weight load enabled.
  - per (nbp, head, m-chunk): S [128m, 1024n] in PSUM -> E =
    exp(S/8 - 8) fp16 (bias keeps both E and unnormalized U inside fp16
    range; softmax shift-invariance + the ones-colsum absorb it) ->
    U[128, 512] x2 += over m.
  - colsums at rows {0,32,64,96} of cs (f32); 1/x via DVE reciprocal,
    broadcast across partitions by an SBUF->SBUF DMA, U scaled by DVE
    mul, out-projected, DMA'd out.

Scheduling: n-block-major loops; all setup (projections, w4) and
epilogue (normalize, out-proj) work is split into ~1-2us units drained
at most one per m-iteration, gated by earliest-safe iteration, so the
tensor engine (~154us of work) never idles and the scalar engine's exp
stream (~143us) runs close behind.  The PE array is warmed with dummy
matmuls during the input DMA so the HAM clock gate is released before
real work starts.
"""

import numpy as np

import concourse.bass as bass
import concourse.bacc as bacc
import concourse.tile as tile
import concourse.mybir as mybir
from concourse.bass_utils import run_bass_kernel_spmd

F32 = mybir.dt.float32
F16 = mybir.dt.float16

B, N, Q, K, P, D = 2, 2048, 1024, 16, 1024, 64
HPC = 4          # heads per core
NCORES = 8
INV_TEMP = 1.0 / 8.0    # 1/sqrt(D)
EXP_BIAS = -8.0

PCH = P // 128   # 8 p-chunks
MCH = N // 128   # 16 m-chunks
QBLK = Q // 512  # 2 q-blocks
NWARM = 28       # PE warmup matmuls during input DMA


def build_program(use_mask: bool):
    nc = bacc.Bacc("TRN2", target_bir_lowering=False, debug=False,
                   num_devices=NCORES)

    yT = nc.dram_tensor("yT", [P, N], F16, kind="ExternalInput").ap()
    lamx = nc.dram_tensor("lamx", [P, HPC * D], F16, kind="ExternalInput").ap()
    lamy = nc.dram_tensor("lamy", [P, HPC * D], F16, kind="ExternalInput").ap()
    thx = nc.dram_tensor("thx", [P, HPC * D], F16, kind="ExternalInput").ap()
    thyT = nc.dram_tensor("thyT", [HPC * D, Q], F16, kind="ExternalInput").ap()
    if use_mask:
        maskd = nc.dram_tensor("mask", [N, N], F32, kind="ExternalInput").ap()
    outp = nc.dram_tensor("outp", [N, Q], F32, kind="ExternalOutput").ap()

    lp = nc.allow_low_precision(reason="fp16 matmul pipeline by design")
    lp.__enter__()
    with tile.TileContext(nc) as tc:
        with (
            tc.tile_pool(name="big8k", bufs=19) as bp,
            tc.tile_pool(name="wpool", bufs=1) as wp,
            tc.tile_pool(name="small", bufs=1) as sp,
            tc.tile_pool(name="epool", bufs=3) as ep,
            tc.tile_pool(name="opool", bufs=2) as op,
            tc.tile_pool(name="ps_s", bufs=2, space="PSUM") as pps,
            tc.tile_pool(name="ps_u", bufs=4, space="PSUM") as ppu,
        ):
            # ---- load inputs (bandwidth-bound ~16-20us): small weights
            # first so projection stationaries are ready, then the yT
            # chunks whose arrivals pace the p-interleaved prelude;
            # thyT (first used by out-proj) last.  Alternate queues. ----
            yt, wx, wy, wt = [], [], [], []
            qi = [0]

            def dma_alt(out, in_):
                eng = nc.gpsimd if qi[0] % 2 == 0 else nc.sync
                qi[0] += 1
                eng.dma_start(out=out, in_=in_)

            for lst, src, nm in ((wx, lamx, "wx"), (wy, lamy, "wy"),
                                 (wt, thx, "wt")):
                for p in range(PCH):
                    t = wp.tile([128, HPC * D], F16, tag=f"{nm}{p}",
                                name=f"{nm}{p}")
                    dma_alt(t, src[p * 128:(p + 1) * 128, :])
                    lst.append(t)
            for p in range(PCH):
                t = bp.tile([128, N], F16, tag="big", name=f"yt{p}")
                dma_alt(t, yT[p * 128:(p + 1) * 128, :])
                yt.append(t)
            ty = []
            for j2 in range(2):
                t = wp.tile([128, Q], F16, tag=f"thyT{j2}", name=f"thyT{j2}")
                nc.sync.dma_start(out=t, in_=thyT[j2 * 128:(j2 + 1) * 128, :])
                ty.append(t)

            # ---- PE warmup: dummy matmuls spanning the input-DMA window
            # keep the HAM clock gate released when real work starts ----
            wsc = sp.tile([128, 512], F16, tag="wsc", name="wsc")
            nc.vector.memset(wsc, 0.0)
            wps = ppu.tile([128, 512], F32, tag="pu", name="wps")
            for _ in range(NWARM):
                nc.tensor.matmul(wps, wsc[:, 0:128], wsc,
                                 start=True, stop=True)

            biasc = sp.tile([128, 1], F32, tag="biasc", name="biasc")
            nc.vector.memset(biasc, EXP_BIAS)

            # colsums: head j at partition 32*j; reciprocals in rc
            cs = sp.tile([128, N], F32, tag="cs", name="cs")
            nc.gpsimd.memset(cs, 1.0)
            rc = sp.tile([128, N], F32, tag="rc", name="rc")

            # ---- persistent fp16 operand tiles (w4 memsets first:
            # w4[0] is needed by the prelude before the r1z zeros are) ----
            # zero-fills on DVE (idle early; gpsimd's queue must stay
            # short so yT DMA issues aren't delayed), ones on gpsimd
            w4 = []
            for m in range(MCH):
                t = sp.tile([128, HPC * 128], F16, tag=f"w4_{m}",
                            name=f"w4_{m}")
                nc.vector.memset(t, 0.0)
                for j in range(HPC):
                    nc.gpsimd.memset(t[:, j * 128 + 64:j * 128 + 65], 1.0)
                w4.append(t)

            r1z = [bp.tile([128, N], F16, tag="big", name=f"r1z{j}")
                   for j in range(HPC)]
            for j in range(HPC):
                po = 64 * (j % 2)
                zo = 64 - po
                nc.vector.memset(r1z[j][zo:zo + 64, :], 0.0)
            sT = [bp.tile([128, N], F16, tag="big", name=f"sT{hp}")
                  for hp in range(2)]
            uta = [bp.tile([128, N], F16, tag="big", name=f"uta{j2}")
                   for j2 in range(2)]

            # ---- work units (each ~1-2us of tensor time) ----
            def proj_unit(dsts, wsrc, hp, mbp, mh):
                """[128, 512] psum column of the r1/s projection."""
                acc = ppu.tile([128, 512], F32, tag="pu", name="acc")
                mb = mbp * 2 + mh
                for p in range(PCH):
                    nc.tensor.matmul(
                        acc,
                        wsrc[p][:, hp * 128:(hp + 1) * 128],
                        yt[p][:, mb * 512:(mb + 1) * 512],
                        start=(p == 0), stop=(p == PCH - 1),
                    )
                msl = slice(mb * 512, (mb + 1) * 512)
                for dst, dr, sr, nr in dsts:
                    nc.vector.tensor_copy(
                        out=dst[dr:dr + nr, msl], in_=acc[sr:sr + nr, :])

            def r1_unit(hp, mbp, mh):
                return lambda: proj_unit(
                    [(r1z[2 * hp], 0, 0, 64), (r1z[2 * hp + 1], 64, 64, 64)],
                    wx, hp, mbp, mh)

            def s_unit(hp, mbp, mh):
                return lambda: proj_unit([(sT[hp], 0, 0, 128)],
                                         wy, hp, mbp, mh)

            def w4_unit(mp):
                """w4 for m-chunk pair (2*mp, 2*mp+1)."""
                def emit():
                    accw = ppu.tile([128, 512], F32, tag="pu", name="accw")
                    for mh in range(2):
                        m = mp * 2 + mh
                        for p in range(PCH):
                            nc.tensor.matmul(
                                accw[:, mh * 256:(mh + 1) * 256],
                                yt[p][:, m * 128:(m + 1) * 128],
                                wt[p][:, :],
                                start=(p == 0), stop=(p == PCH - 1),
                            )
                    for mh in range(2):
                        m = mp * 2 + mh
                        for j in range(HPC):
                            nc.vector.tensor_copy(
                                out=w4[m][:, j * 128:j * 128 + 64],
                                in_=accw[:, mh * 256 + j * 64:
                                         mh * 256 + (j + 1) * 64])
                return emit

            ones64 = sp.tile([1, 64], F16, tag="ones64", name="ones64")
            nc.vector.memset(ones64, 1.0)

            def norm_unit(j, nh, nbp, tail=False):
                """scale uta[head j, 512-wide n-block] by 1/colsum,
                partition-broadcast via a ones-vector matmul."""
                def emit():
                    hp, po = j // 2, 64 * (j % 2)
                    n0 = nbp * 1024 + nh * 512
                    nsl = slice(n0, n0 + 512)
                    rt = sp.tile([1, 512], F16, tag="rt", bufs=4, name="rt")
                    nc.vector.tensor_copy(out=rt, in_=rc[32 * j:32 * j + 1,
                                                         nsl])
                    pb = ppu.tile([128, 512], F32, tag="pu", name="pb")
                    nc.tensor.matmul(pb[0:64, :], ones64, rt,
                                     start=True, stop=True)
                    nc.vector.tensor_mul(
                        uta[hp][po:po + 64, nsl],
                        uta[hp][po:po + 64, nsl],
                        pb[0:64, :],
                    )
                return emit

            def outproj_unit(nch, qb, tail):
                def emit():
                    po_ = ppu.tile([128, 512], F32, tag="pu", name="po_")
                    for j2 in range(2):
                        nc.tensor.matmul(
                            po_,
                            uta[j2][:, nch * 128:(nch + 1) * 128],
                            ty[j2][:, qb * 512:(qb + 1) * 512],
                            start=(j2 == 0), stop=(j2 == 1),
                        )
                    osb = op.tile([128, 512], F32, tag="osb", bufs=4,
                                  name="osb")
                    # tail: alternate ACT/DVE (both idle, halves the
                    # serialized copy chain)
                    if tail and (nch + qb) % 2 == 0:
                        nc.scalar.copy(out=osb, in_=po_)
                    else:
                        nc.vector.tensor_copy(out=osb, in_=po_)
                    deng = nc.gpsimd if (nch + qb) % 2 == 0 else nc.sync
                    deng.dma_start(
                        out=outp[nch * 128:(nch + 1) * 128,
                                 qb * 512:(qb + 1) * 512],
                        in_=osb)
                return emit

            def outproj_units(nbp, tail):
                units = []
                for nh in range(2):
                    for nch in range(nbp * 8 + nh * 4, nbp * 8 + nh * 4 + 4):
                        for qb in range(QBLK):
                            units.append(outproj_unit(nch, qb, tail))
                return units

            # deferred work: min-heap of (earliest_iter, seq, emit_fn)
            import heapq
            queue = []
            qseq = [0]

            def qpush(earliest, fn):
                heapq.heappush(queue, (earliest, qseq[0], fn))
                qseq[0] += 1

            for earliest, fn in [
                (0, w4_unit(1)), (2, r1_unit(0, 0, 1)), (2, w4_unit(2)),
                (4, w4_unit(3)), (5, r1_unit(0, 1, 0)), (6, w4_unit(4)),
                (8, w4_unit(5)), (8, r1_unit(0, 1, 1)), (10, w4_unit(6)),
                (12, w4_unit(7)),
                (16, r1_unit(1, 0, 0)), (19, r1_unit(1, 0, 1)),
                (22, r1_unit(1, 1, 0)), (25, r1_unit(1, 1, 1)),
                (27, s_unit(1, 0, 0)), (29, s_unit(1, 0, 1)),
                (40, s_unit(0, 1, 0)), (46, s_unit(0, 1, 1)),
                (52, s_unit(1, 1, 0)), (58, s_unit(1, 1, 1)),
            ]:
                qpush(earliest, fn)

            # ---- prelude: minimum to start (nbp 0, head 0, m = 0),
            # p-interleaved so each landing yT chunk immediately feeds
            # all four accumulators instead of serializing per unit ----
            r1_unit(0, 0, 0)()
            s_unit(0, 0, 0)()
            s_unit(0, 0, 1)()
            w4_unit(0)()

            # ---- main stream: scores -> exp -> aggregate ----
            for nbp in range(2):
                n0 = nbp * 1024
                for j in range(HPC):
                    hp, po = j // 2, 64 * (j % 2)
                    pua = ppu.tile([128, 512], F32, tag="pu", name="pua")
                    pub = ppu.tile([128, 512], F32, tag="pu", name="pub")
                    for m in range(MCH):
                        it = (nbp * HPC + j) * MCH + m
                        ps = pps.tile([128, 1024], F32, tag="ps", name="ps")
                        for half in range(2):
                            nc.tensor.matmul(
                                ps[:, half * 512:(half + 1) * 512],
                                r1z[j][:, m * 128:(m + 1) * 128],
                                sT[hp][:, n0 + half * 512:
                                       n0 + (half + 1) * 512],
                                start=True, stop=True,
                            )
                        if use_mask:
                            mt = op.tile([128, 1024], F32, tag="mt",
                                         name="mt")
                            nc.gpsimd.dma_start(
                                out=mt,
                                in_=maskd[m * 128:(m + 1) * 128,
                                          n0:n0 + 1024])
                            nc.vector.tensor_add(ps, ps, mt)
                        e = ep.tile([128, 1024], F16, tag="e", name="e")
                        nc.scalar.activation(
                            out=e, in_=ps,
                            func=mybir.ActivationFunctionType.Exp,
                            scale=INV_TEMP, bias=biasc,
                        )
                        nc.tensor.matmul(
                            pua, w4[m][:, j * 128:(j + 1) * 128],
                            e[:, 0:512],
                            start=(m == 0), stop=(m == MCH - 1))
                        nc.tensor.matmul(
                            pub, w4[m][:, j * 128:(j + 1) * 128],
                            e[:, 512:1024],
                            start=(m == 0), stop=(m == MCH - 1))
                        if queue and queue[0][0] <= it:
                            heapq.heappop(queue)[2]()
                    last = (j == HPC - 1 and nbp == 1)
                    for half, pu in ((0, pua), (1, pub)):
                        nsl = slice(n0 + half * 512, n0 + (half + 1) * 512)
                        nc.vector.tensor_copy(
                            out=cs[32 * j:32 * j + 1, nsl], in_=pu[64:65, :])
                    # this head's colsums final: reciprocal (approx_fast,
                    # ~18 correct bits; colsums are well-conditioned
                    # positive normals).  Full 128-partition APs: the
                    # custom DVE op misbehaves on 1-partition slices;
                    # other heads' rows hold placeholder 1.0s that their
                    # own later recip overwrites before use.
                    for ch in range(2):
                        hsl = slice(n0 + ch * 512, n0 + (ch + 1) * 512)
                        nc.vector.reciprocal_approx_fast(
                            out=rc[:, hsl], in_=cs[:, hsl])
                    for half, pu in ((0, pua), (1, pub)):
                        nsl = slice(n0 + half * 512, n0 + (half + 1) * 512)
                        if last:
                            # scalar engine is idle after the final exp;
                            # keep the tail's DVE queue short
                            nc.scalar.copy(out=uta[hp][po:po + 64, nsl],
                                           in_=pu[0:64, :])
                        else:
                            nc.vector.tensor_copy(
                                out=uta[hp][po:po + 64, nsl], in_=pu[0:64, :])
                    # this head's normalization can run as soon as its
                    # block is done; only head 3's stays in the tail
                    blk_end = (nbp * HPC + j) * MCH + MCH - 1
                    if not last:
                        for nh in range(2):
                            qpush(blk_end + 3 + nh, norm_unit(j, nh, nbp))
                    if j == HPC - 1 and nbp == 0:
                        for i, u in enumerate(outproj_units(0, False)):
                            qpush(80 + 2 * i, u)
            # ---- tail: head 3 nbp1 normalization + nbp1 out-projection.
            # A few dependency-free filler matmuls keep the HAM clock gate
            # released while the short DVE chain (recip/rt/mul) resolves.
            while queue:
                heapq.heappop(queue)[2]()
            wps2 = ppu.tile([128, 512], F32, tag="pu", name="wps2")
            for _ in range(6):
                nc.tensor.matmul(wps2, wsc[:, 0:128], wsc,
                                 start=True, stop=True)
            norm_unit(HPC - 1, 0, 1)()
            norm_unit(HPC - 1, 1, 1)()
            for u in outproj_units(1, True):
                u()

    lp.__exit__(None, None, None)
    nc.compile()
    return nc


_PROG_CACHE = {}


def _get_program(use_mask: bool):
    if use_mask not in _PROG_CACHE:
        _PROG_CACHE[use_mask] = build_program(use_mask)
    return _PROG_CACHE[use_mask]


def make_in_maps(y_prime, mask, Lam_x, Lam_y, Th_x, Th_y, use_mask):
    in_maps = []
    for c in range(NCORES):
        b = c // 4
        heads = [4 * (c % 4) + j for j in range(HPC)]
        m = {
            "yT": np.ascontiguousarray(y_prime[b].T).astype(np.float16),
            "lamx": np.ascontiguousarray(
                Lam_x[heads].transpose(1, 0, 2).reshape(P, HPC * D)
            ).astype(np.float16),
            "lamy": np.ascontiguousarray(
                Lam_y[heads].transpose(1, 0, 2).reshape(P, HPC * D)
            ).astype(np.float16),
            "thx": np.ascontiguousarray(
                Th_x[heads].transpose(1, 0, 2).reshape(P, HPC * D)
            ).astype(np.float16),
            "thyT": np.ascontiguousarray(
                Th_y[heads].transpose(0, 2, 1).reshape(HPC * D, Q)
            ).astype(np.float16),
        }
        if use_mask:
            m["mask"] = np.ascontiguousarray(mask).astype(np.float32)
        in_maps.append(m)
    return in_maps


def kernel(y_prime, mask, Lam_x, Lam_y, Th_x, Th_y, _trace=False):
    y_prime = np.asarray(y_prime, dtype=np.float32)
    mask = np.asarray(mask, dtype=np.float32)
    Lam_x = np.asarray(Lam_x, dtype=np.float32)
    Lam_y = np.asarray(Lam_y, dtype=np.float32)
    Th_x = np.asarray(Th_x, dtype=np.float32)
    Th_y = np.asarray(Th_y, dtype=np.float32)

    use_mask = bool(np.any(mask))
    nc = _get_program(use_mask)
    in_maps = make_in_maps(y_prime, mask, Lam_x, Lam_y, Th_x, Th_y, use_mask)
    r = run_bass_kernel_spmd(nc, in_maps, core_ids=list(range(NCORES)),
                             trace=_trace)
    out = np.zeros((B, N, Q), dtype=np.float32)
    for c in range(NCORES):
        out[c // 4] += r.results[c]["outp"]
    if _trace:
        kernel.last_results = r
    return out
